# revision 7
# baseline (speedup 1.0000x reference)
"""Trainium2 Bass kernel for masked attention with score_adjust.

reference computation (per batch b, head h):
    scores = (Q @ K^T + score_adjust) / sqrt(D)
    scores = where(mask, -inf, scores)          # mask[b,0] shared over heads
    p_attn = softmax(scores, axis=-1)
    out    = p_attn @ V
returns (out, p_attn), both fp32.

Sharding: B*H = 32 (batch, head) pairs across 8 cores -> 4 pairs/core,
each core handles 4 heads of a single batch. Pure data parallelism.

Host-side prep: madj = score_adjust + (-2000 where masked). After the
1/sqrt(D) scale the masked scores are <= -240, and exp underflows to
exactly 0 in fp32 -- so masked positions drop out of softmax with no
on-chip mask handling at all (matches exp(-inf) = 0; we also skip the
usual max-subtraction since unmasked scores are bounded ~|10|, far from
fp32 exp overflow).

Per-core kernel (per pair, per 128-row query tile):
  - PE: scores = Q K^T into PSUM (lhsT = Q^T built via PE transpose at
        setup), then += madj via an identity-weight matmul accumulate.
  - ACT: e = exp(scale * scores) PSUM->SBUF, with fused row-sum accum.
  - DVE: recip = 1/sums; p_norm = e * recip -> DMA to p_attn output.
  - PE: transpose e 128x128 blocks, PV matmul accumulate over key chunks.
  - DVE: out = psum_pv * recip -> DMA out.

Self-contained: hardcodes shapes; no file reads.
"""

import numpy as np

import concourse.bass as bass
import concourse.mybir as mybir
import concourse.tile as tile
from concourse import bacc
from concourse.bass_utils import run_bass_kernel_spmd
from concourse.masks import make_identity

F32 = mybir.dt.float32
F32R = mybir.dt.float32r
BF16 = mybir.dt.bfloat16
AF = mybir.ActivationFunctionType
ALU = mybir.AluOpType

B, H, S, D = 2, 16, 2048, 64
N_CORES = 8
PAIRS = (B * H) // N_CORES  # 4 pairs per core
SCALE = 1.0 / float(np.sqrt(D))
MASK_BIAS = -2000.0

# config: dtype/path choices (tuned after HW measurement)
CFG = {
    "qk_dtype": "f32r",   # 'f32' | 'f32r'   dtype for QK^T + adj matmuls
    "pv_dtype": "bf16",   # 'f32' | 'bf16'   dtype for P transposes + PV matmul
}


def build_attention(nc, tc, cfg, n_pairs, s, d):
    """Emit the attention body into TileContext tc."""
    P = 128
    nt = s // P            # query tiles per pair
    nsl = max(1, s // 512)  # 512-wide score slices
    sl = s // nsl
    qk_r = cfg["qk_dtype"] == "f32r"
    qk_dt = F32R if qk_r else F32
    pv_bf = cfg["pv_dtype"] == "bf16"
    pv_dt = BF16 if pv_bf else F32

    q_h = nc.dram_tensor("q", (n_pairs, s, d), F32, kind="ExternalInput").ap()
    k_h = nc.dram_tensor("k", (n_pairs, s, d), F32, kind="ExternalInput").ap()
    v_h = nc.dram_tensor("v", (n_pairs, s, d), F32, kind="ExternalInput").ap()
    # madj bytes are plain fp32; declared f32r so the PE consumes them in
    # fast mode directly (PE rounds internally; no producer-side rounding).
    adj_h = nc.dram_tensor("adj", (n_pairs, s, s), qk_dt, kind="ExternalInput").ap()
    out_h = nc.dram_tensor("out_o", (n_pairs, s, d), F32, kind="ExternalOutput").ap()
    p_h = nc.dram_tensor("p_o", (n_pairs, s, s), F32, kind="ExternalOutput").ap()

    import contextlib
    ctx = contextlib.ExitStack()
    with ctx:
        singles = ctx.enter_context(tc.tile_pool(name="singles", bufs=1))
        pair_pool = ctx.enter_context(tc.tile_pool(name="pair", bufs=2))
        adj_pool = ctx.enter_context(tc.tile_pool(name="adjp", bufs=3))
        e_pool = ctx.enter_context(tc.tile_pool(name="ep", bufs=2))
        p_pool = ctx.enter_context(tc.tile_pool(name="pp", bufs=2))
        pn_pool = ctx.enter_context(tc.tile_pool(name="pnp", bufs=2))
        pt_pool = ctx.enter_context(tc.tile_pool(name="ptp", bufs=2))
        small_pool = ctx.enter_context(tc.tile_pool(name="smalls", bufs=3))
        psum_s = ctx.enter_context(tc.tile_pool(name="psums", bufs=1, space="PSUM"))
        psum_t = ctx.enter_context(tc.tile_pool(name="psumt", bufs=2, space="PSUM"))
        psum_o = ctx.enter_context(tc.tile_pool(name="psumo", bufs=2, space="PSUM"))

        # constants
        ident = singles.tile([P, P], F32, name="ident", tag="ident")
        make_identity(nc, ident)
        if pv_bf:
            ident_pv = singles.tile([P, P], BF16, name="ident_pv", tag="ident_pv")
            nc.vector.tensor_copy(ident_pv, ident)
        else:
            ident_pv = ident
        if qk_r:
            ident_mm = singles.tile([P, P], F32R, name="ident_r", tag="ident_r")
            nc.vector.tensor_copy(ident_mm, ident)
        else:
            ident_mm = ident

        for i in range(n_pairs):
            # ---- per-pair setup: load q/k/v, build qT [d, s], kT [d, s] ----
            q_sb = pair_pool.tile([P, nt, d], F32, name="q_sb", tag="q_sb")
            nc.sync.dma_start(q_sb, q_h[i].rearrange("(t p) d -> p t d", p=P))
            k_sb = pair_pool.tile([P, nt, d], F32, name="k_sb", tag="k_sb")
            nc.sync.dma_start(k_sb, k_h[i].rearrange("(t p) d -> p t d", p=P))
            v_sb = pair_pool.tile([P, nt, d], pv_dt, name="v_sb", tag="v_sb")
            if pv_bf:
                nc.gpsimd.dma_start(v_sb, v_h[i].rearrange("(t p) d -> p t d", p=P))
            else:
                nc.sync.dma_start(v_sb, v_h[i].rearrange("(t p) d -> p t d", p=P))

            qT = pair_pool.tile([d, s], qk_dt, name="qT", tag="qT")
            kT = pair_pool.tile([d, s], qk_dt, name="kT", tag="kT")
            for src, dst in ((q_sb, qT), (k_sb, kT)):
                for g in range(nt // 4):
                    ps = psum_t.tile([P, 512], F32, name="ps_setup", tag="pT")
                    for j in range(4):
                        t = 4 * g + j
                        nc.tensor.transpose(
                            ps[:d, j * P:(j + 1) * P], src[:, t, :], ident
                        )
                    nc.any.tensor_copy(dst[:, g * 512:(g + 1) * 512], ps[:d, :])

            # ---- main loop over query tiles ----
            for t in range(nt):
                adj_t = adj_pool.tile([P, s], qk_dt, name="adj_t", tag="adj_t")
                nc.sync.dma_start(adj_t, adj_h[i, t * P:(t + 1) * P, :])

                ps_s = psum_s.tile([P, s], F32, name="ps_s", tag="s")
                for j in range(nsl):
                    c0 = j * sl
                    nc.tensor.matmul(
                        ps_s[:, c0:c0 + sl],
                        lhsT=qT[:, t * P:(t + 1) * P],
                        rhs=kT[:, c0:c0 + sl],
                        start=True,
                        stop=False,
                    )
                    nc.tensor.matmul(
                        ps_s[:, c0:c0 + sl],
                        lhsT=ident_mm,
                        rhs=adj_t[:, c0:c0 + sl],
                        start=False,
                        stop=True,
                    )

                # e = exp(scale * scores), fused row sums
                e_t = e_pool.tile([P, s], F32, name="e_t", tag="e_t")
                sums = small_pool.tile([P, 1], F32, name="sums", tag="sums")
                nc.scalar.activation(e_t, ps_s, AF.Exp, scale=SCALE, accum_out=sums)
                recip = small_pool.tile([P, 1], F32, name="recip", tag="recip")
                nc.vector.reciprocal(recip, sums)

                # normalized p for the p_attn output (fp32)
                pn_t = pn_pool.tile([P, s], F32, name="pn_t", tag="pn_t")
                nc.vector.tensor_scalar_mul(pn_t, e_t, recip)
                nc.sync.dma_start(p_h[i, t * P:(t + 1) * P, :], pn_t)

                # transposes of (unnormalized) p for the PV matmul
                if pv_bf:
                    p_t = p_pool.tile([P, s], BF16, name="p_t", tag="p_t")
                    nc.vector.tensor_copy(p_t, e_t)
                else:
                    p_t = e_t
                pt_sb = pt_pool.tile([P, nt * P], pv_dt, name="pt_sb", tag="pt_sb")
                for g in range(nt // 4):
                    ps_tr = psum_t.tile([P, 512], pv_dt, name="ps_tr", tag="pT")
                    for j in range(4):
                        c = 4 * g + j
                        nc.tensor.transpose(
                            ps_tr[:, j * P:(j + 1) * P],
                            p_t[:, c * P:(c + 1) * P],
                            ident_pv,
                        )
                    nc.scalar.copy(pt_sb[:, g * 512:(g + 1) * 512], ps_tr)

                # PV matmul accumulate over key chunks
                ps_pv = psum_o.tile([P, d], F32, name="ps_pv", tag="o")
                for c in range(nt):
                    nc.tensor.matmul(
                        ps_pv,
                        lhsT=pt_sb[:, c * P:(c + 1) * P],
                        rhs=v_sb[:, c, :],
                        start=(c == 0),
                        stop=(c == nt - 1),
                    )
                out_t = small_pool.tile([P, d], F32, name="out_t", tag="out_t")
                nc.vector.tensor_scalar_mul(out_t, ps_pv, recip)
                nc.sync.dma_start(out_h[i, t * P:(t + 1) * P, :], out_t)


def build(cfg=None, n_pairs=PAIRS, s=S, d=D, debug=False):
    cfg = dict(CFG, **(cfg or {}))
    nc = bacc.Bacc("TRN2", target_bir_lowering=False, debug=debug)
    with tile.TileContext(nc) as tc:
        build_attention(nc, tc, cfg, n_pairs, s, d)
    nc.compile()
    return nc


_NC_CACHE = {}


def _get_nc():
    key = tuple(sorted(CFG.items()))
    if key not in _NC_CACHE:
        _NC_CACHE[key] = build()
    return _NC_CACHE[key]


def make_in_maps(query, key, value, mask, score_adjust):
    mb = [np.where(mask[b, 0], np.float32(MASK_BIAS), np.float32(0.0))
          for b in range(B)]
    in_maps = []
    for c in range(N_CORES):
        b = c // (N_CORES // B)
        h0 = (c % (N_CORES // B)) * PAIRS
        in_maps.append({
            "q": np.ascontiguousarray(query[b, h0:h0 + PAIRS]),
            "k": np.ascontiguousarray(key[b, h0:h0 + PAIRS]),
            "v": np.ascontiguousarray(value[b, h0:h0 + PAIRS]),
            "adj": score_adjust[b, h0:h0 + PAIRS] + mb[b][None, :, :],
        })
    return in_maps


def assemble(results):
    out = np.empty((B, H, S, D), np.float32)
    p = np.empty((B, H, S, S), np.float32)
    for c in range(N_CORES):
        b = c // (N_CORES // B)
        h0 = (c % (N_CORES // B)) * PAIRS
        out[b, h0:h0 + PAIRS] = results[c]["out_o"]
        p[b, h0:h0 + PAIRS] = results[c]["p_o"]
    return out, p


def kernel(query, key, value, mask, score_adjust):
    query = np.asarray(query, dtype=np.float32)
    key = np.asarray(key, dtype=np.float32)
    value = np.asarray(value, dtype=np.float32)
    score_adjust = np.asarray(score_adjust, dtype=np.float32)
    mask = np.asarray(mask)
    nc = _get_nc()
    in_maps = make_in_maps(query, key, value, mask, score_adjust)
    res = run_bass_kernel_spmd(nc, in_maps, core_ids=list(range(N_CORES)))
    return assemble(res.results)
